# revision 2
# baseline (speedup 1.0000x reference)
"""DeFeat distillation loss on 8 Trainium2 NeuronCores (Bass/Tile), v2.

Data-parallel over the batch dim (B=8 -> 1 batch element per core).

The loss only needs TWO scalars per pyramid level (masked / unmasked sums
of (t - Ws)^2), so the host permutes each level's pixels mask-first and
zero-pads both runs to a static block grid.  Every device-side reduction
then covers a whole block (one accum column per block) and the host maps
blocks -> masked/background sums afterwards.  No per-pixel output.

Layout is channel-major ("flipped"): psum[oc, px].  Per block of W pixels:
  psum[0:128, 0:W]   = 16W @ 16s   (oc 0:128)    via ONE DoubleRow fp8
  psum[0:128, W:2W]  = ...         (oc 128:256)  matmul per 512-px sub
(K=256 contraction in one pass; the adaptation weights are the stationary
operand so LDWEIGHTS hides under the moving stream).  The teacher
subtract is a -16I matmul (moving = fp8 t, natural channel-major layout)
on ~5/8 of the blocks; the rest subtract on the DVE (stt).  The square +
reduce is a single [128, 2W] Square+accum per block, routed ACT (from
psum or from the DVE's bf16 d) or DVE (from bf16 d) to balance engines.

fp8 scaling as v1: s, W, t' = 16*(t-b) all fp8e4m3 x16; psum holds 256*d;
ACT applies scale=1/256 inside Square so every accum column is sum(d^2)
exactly.  Host sums columns, splits masked/bg per level, sqrt in f64.
"""

import os
import sys

for _p in ("/opt/trn_rl_repo", os.path.expanduser("~/.axon_site/_ro/trn_rl_repo")):
    if os.path.isdir(_p) and _p not in sys.path:
        sys.path.insert(0, _p)

import numpy as np
import ml_dtypes

F8 = ml_dtypes.float8_e4m3
S_SCALE = 16.0

WEIGHT_GT = 0.004
WEIGHT_BG = 0.0002
STRIDES = (8, 16, 32, 64, 128)
SIZES = (128, 64, 32, 16, 8)
HWS = tuple(s * s for s in SIZES)          # (16384, 4096, 1024, 256, 64)
B, C, NBOX = 8, 256, 16
N_CORES = 8
N_LEVELS = 5

# Static block grid: (level, W) per block.  Per level the host lays the
# permuted pixel stream out as [masked | pad | unmasked | pad | zeros].
BLK_W = (1024, 1024, 512, 256, 64)         # block width per level
BLK_N = (17, 5, 3, 2, 2)                   # blocks per level
BLOCKS = [(l, BLK_W[l]) for l in range(N_LEVELS) for _ in range(BLK_N[l])]
NBLK = len(BLOCKS)                         # 29
LVL_B0 = [sum(BLK_N[:l]) for l in range(N_LEVELS)]

# Per-block route: "A"  = PE negI subtract + ACT Square from psum
#                  "Ba" = DVE stt subtract + ACT Square from bf16 d
#                  "Bd" = DVE stt subtract + DVE square from bf16 d
# Cycle approximates A=5/8, Ba=1/8, Bd=2/8 (engine balance, see module doc).
_CYCLE = ("A", "Bd", "A", "A", "Ba", "A", "Bd", "A")
ROUTES = [_CYCLE[i % len(_CYCLE)] for i in range(NBLK)]


def _build_module():
    import concourse.mybir as mybir
    from concourse import bacc
    from concourse.tile import TileContext

    dt = mybir.dt
    nc = bacc.Bacc("TRN2", target_bir_lowering=False, debug=False,
                   num_devices=N_CORES)

    blk_d = [nc.dram_tensor(f"blk{k}", [128, 4, w], dt.float8e4,
                            kind="ExternalInput")
             for k, (_, w) in enumerate(BLOCKS)]
    wt_d = nc.dram_tensor("wt", [128, 4 * N_LEVELS, 128], dt.float8e4,
                          kind="ExternalInput")
    ni_d = nc.dram_tensor("ni", [128, 128], dt.float8e4, kind="ExternalInput")
    out_q = nc.dram_tensor("out_q", [128, NBLK], dt.float32,
                           kind="ExternalOutput")

    SUB = mybir.AluOpType.subtract
    BYP = mybir.AluOpType.bypass
    MULT = mybir.AluOpType.mult
    SQUARE = mybir.ActivationFunctionType.Square
    DR = mybir.MatmulPerfMode.DoubleRow

    with TileContext(nc) as tc:
        with (
            tc.tile_pool(name="const", bufs=1) as const_pool,
            tc.tile_pool(name="feat", bufs=8) as feat_pool,
            tc.tile_pool(name="work", bufs=3) as work_pool,
            tc.tile_pool(name="ps", bufs=2, space="PSUM") as psum_pool,
        ):
            wt = const_pool.tile([128, 4 * N_LEVELS, 128], dt.float8e4)
            ni = const_pool.tile([128, 128], dt.float8e4)
            qcat = const_pool.tile([128, NBLK], dt.float32)
            garb_a = const_pool.tile([128, 2048], dt.bfloat16)
            garb_v = const_pool.tile([128, 2048], dt.bfloat16)

            nc.sync.dma_start(out=wt[:], in_=wt_d[:])
            nc.sync.dma_start(out=ni[:], in_=ni_d[:])

            for k, (lvl, w) in enumerate(BLOCKS):
                route = ROUTES[k]
                blk = feat_pool.tile([128, 4, w], dt.float8e4, tag="blk")
                nc.sync.dma_start(out=blk[:], in_=blk_d[k][:])

                ps = psum_pool.tile([128, 2048], dt.float32, tag="ps")
                nsub = (w + 511) // 512
                # adaptation matmuls, grouped by stationary (oc chunk)
                for j in range(2):
                    for u in range(nsub):
                        sw = min(512, w - 512 * u)
                        c0 = j * w + 512 * u
                        nc.tensor.matmul(
                            ps[:, c0:c0 + sw],
                            wt[:, 4 * lvl + 2 * j:4 * lvl + 2 * j + 2, :],
                            blk[:, 0:2, 512 * u:512 * u + sw],
                            start=True, stop=(route != "A"), perf_mode=DR)
                if route == "A":
                    # psum -= 16 * t'' via -16I matmul (one per 512-px sub)
                    for j in range(2):
                        for u in range(nsub):
                            sw = min(512, w - 512 * u)
                            c0 = j * w + 512 * u
                            nc.tensor.matmul(
                                ps[:, c0:c0 + sw],
                                ni[:],
                                blk[:, 2 + j, 512 * u:512 * u + sw],
                                start=False, stop=True)
                    nc.scalar.activation(
                        garb_a[:, 0:2 * w], ps[:, 0:2 * w], SQUARE,
                        scale=1.0 / 256.0, accum_out=qcat[:, k:k + 1])
                else:
                    d = work_pool.tile([128, 2048], dt.bfloat16, tag="d")
                    # d = 16*t'' - psum  (= 256*(t' - a))
                    nc.vector.scalar_tensor_tensor(
                        d[:, 0:2 * w], blk[:, 2:4, :], S_SCALE,
                        ps[:, 0:2 * w], op0=MULT, op1=SUB)
                    if route == "Ba":
                        nc.scalar.activation(
                            garb_a[:, 0:2 * w], d[:, 0:2 * w], SQUARE,
                            scale=1.0 / 256.0, accum_out=qcat[:, k:k + 1])
                    else:
                        nc.vector.scalar_tensor_tensor(
                            garb_v[:, 0:2 * w], d[:, 0:2 * w],
                            1.0 / 65536.0, d[:, 0:2 * w],
                            op0=MULT, op1=MULT,
                            accum_out=qcat[:, k:k + 1])

            nc.sync.dma_start(out=out_q[:], in_=qcat[:])

    nc.compile()
    return nc


def _rasterize_masks(gt_bboxes):
    """Host-side mask rasterization, mirroring reference.gt_mask.

    Returns per-level [B, HW] bool masks."""
    out = []
    for lvl in range(N_LEVELS):
        h = w = SIZES[lvl]
        stride = np.float32(STRIDES[lvl])
        q = np.floor(gt_bboxes.astype(np.float32) / stride).astype(np.int32)
        lx = np.minimum(q[..., 0], w - 1)
        ly = np.minimum(q[..., 1], h - 1)
        rx = np.minimum(q[..., 2], w - 1)
        ry = np.minimum(q[..., 3], h - 1)
        lm = np.zeros((B, h * w), bool)
        for b in range(B):
            m = np.zeros((h, w), bool)
            for i in range(gt_bboxes.shape[1]):
                if lx[b, i] == rx[b, i] or ly[b, i] == ry[b, i]:
                    m[ly[b, i], lx[b, i]] = True
                else:
                    m[ly[b, i]:ry[b, i], lx[b, i]:rx[b, i]] = True
            lm[b] = m.reshape(-1)
        out.append(lm)
    return out


_NC_CACHE = None


def _get_nc():
    global _NC_CACHE
    if _NC_CACHE is None:
        _NC_CACHE = _build_module()
    return _NC_CACHE


def _run(in_maps, trace=False, trace_cores=None):
    from concourse.bass_utils import run_bass_kernel_spmd

    kwargs = {}
    if trace:
        kwargs.update(trace=True, trace_cores=trace_cores or [0])
    return run_bass_kernel_spmd(_get_nc(), in_maps, core_ids=list(range(N_CORES)),
                                **kwargs)


def _pack_wt(inputs):
    """wt[p, 4l+2j+i, m] = 16 * W_l[128j + m, 128i + p]."""
    wtp = np.zeros((128, 4 * N_LEVELS, 128), np.float32)
    for lvl in range(N_LEVELS):
        w = np.asarray(inputs[f"adapt_w{lvl}"], np.float32)
        for j in range(2):
            for i in range(2):
                wtp[:, 4 * lvl + 2 * j + i, :] = \
                    w[128 * j:128 * j + 128, 128 * i:128 * i + 128].T
    return (wtp * S_SCALE).astype(F8)


def _prep_in_maps(inputs, masks):
    """Per-core block arrays [128, 4, W]: [s_ic0 | s_ic1 | t_oc0 | t_oc1],
    pixels permuted mask-first and zero-padded to the static block grid.
    Returns (in_maps, mblocks[B][L]) where mblocks = #masked blocks."""
    wtp = _pack_wt(inputs)
    negi = (-S_SCALE * np.eye(128, dtype=np.float32)).astype(F8)
    mblocks = [[0] * N_LEVELS for _ in range(N_CORES)]
    in_maps = []
    for b in range(N_CORES):
        m = {"wt": wtp, "ni": negi}
        for lvl in range(N_LEVELS):
            hw, g, nb = HWS[lvl], BLK_W[lvl], BLK_N[lvl]
            s = np.asarray(inputs[f"feat_s{lvl}"][b], np.float32).reshape(C, hw)
            bv = np.asarray(inputs[f"adapt_b{lvl}"], np.float32)
            t = np.asarray(inputs[f"feat_t{lvl}"][b], np.float32).reshape(C, hw)
            tp = t - bv[:, None]
            mask = masks[lvl][b]
            midx = np.flatnonzero(mask)
            uidx = np.flatnonzero(~mask)
            nm = len(midx)
            mb = -(-nm // g)                      # ceil
            mblocks[b][lvl] = mb
            tot = nb * g
            st = np.zeros((4, 128, tot), np.float32)
            for (arr, base) in ((s, 0), (tp, 2)):
                sc = arr * S_SCALE
                pm = sc[:, midx]
                pu = sc[:, uidx]
                st[base + 0, :, 0:nm] = pm[0:128]
                st[base + 1, :, 0:nm] = pm[128:256]
                st[base + 0, :, mb * g:mb * g + len(uidx)] = pu[0:128]
                st[base + 1, :, mb * g:mb * g + len(uidx)] = pu[128:256]
            st8 = st.astype(F8)
            k0 = LVL_B0[lvl]
            for i in range(nb):
                m[f"blk{k0 + i}"] = np.ascontiguousarray(
                    st8[:, :, i * g:(i + 1) * g].transpose(1, 0, 2))
        in_maps.append(m)
    return in_maps, mblocks


def kernel(_trace=False, _return_results=False, **inputs):
    gt_bboxes = np.asarray(inputs["gt_bboxes"], np.float32)
    masks = _rasterize_masks(gt_bboxes)
    in_maps, mblocks = _prep_in_maps(inputs, masks)

    res = _run(in_maps, trace=_trace)

    loss = np.float64(0.0)
    for lvl in range(N_LEVELS):
        s_gt = np.float64(0.0)
        s_bg = np.float64(0.0)
        k0, nb = LVL_B0[lvl], BLK_N[lvl]
        for c in range(N_CORES):
            q = res.results[c]["out_q"].astype(np.float64)
            qb = q[:, k0:k0 + nb].sum(axis=0)
            mb = mblocks[c][lvl]
            s_gt += qb[:mb].sum()
            s_bg += qb[mb:].sum()
        loss += WEIGHT_GT * np.sqrt(s_gt + 1e-8) + \
            WEIGHT_BG * np.sqrt(s_bg + 1e-8)

    out = np.array(loss, dtype=np.float32)
    if _return_results:
        return out, res
    return out


# revision 3
# speedup vs baseline: 1.2285x; 1.2285x over previous
"""DeFeat distillation loss on 8 Trainium2 NeuronCores (Bass/Tile), v3.

Data-parallel over the batch dim (B=8 -> 1 batch element per core).

The loss only needs TWO scalars per pyramid level (masked / unmasked sums
of (t - Ws)^2), so the host permutes each level's pixels mask-first and
zero-pads both runs to a static block grid.  Every device-side reduction
then covers a whole block (one accum column per block) and the host maps
blocks -> masked/background sums afterwards.  No per-pixel output.

Layout is channel-major ("flipped"): psum[oc, px].  Per 512-px block:
  psum[0:128, 0:512]    = 16W @ 16s   (oc 0:128)   one DoubleRow fp8
  psum[0:128, 512:1024] = ...         (oc 128:256) matmul each (K=256)
The adaptation weights are the stationary operand so LDWEIGHTS hides
under the moving stream.  The teacher subtract is a -16I matmul (moving
= fp8 t, natural channel-major layout) on ~2/3 of the blocks; the rest
subtract on the DVE (stt).  The square + reduce is a single [128, 2W]
Square+accum per block, routed ACT (psum src) or DVE (bf16 d src).

512-px blocks keep psum tiles at 2 banks -> 4 blocks in flight, which
keeps the PE streaming (HAM stays warm) while ACT/DVE drain.  Blocks are
fetched 4 per DMA (~1 MB transfers) for near-peak HBM bandwidth.

fp8 scaling as v1: s, W, t' = 16*(t-b) all fp8e4m3 x16; psum holds 256*d;
ACT applies scale=1/256 inside Square so every accum column is sum(d^2)
exactly.  Host sums columns, splits masked/bg per level, sqrt in f64.
"""

import os
import sys

for _p in ("/opt/trn_rl_repo", os.path.expanduser("~/.axon_site/_ro/trn_rl_repo")):
    if os.path.isdir(_p) and _p not in sys.path:
        sys.path.insert(0, _p)

import numpy as np
import ml_dtypes

F8 = ml_dtypes.float8_e4m3
S_SCALE = 16.0

WEIGHT_GT = 0.004
WEIGHT_BG = 0.0002
STRIDES = (8, 16, 32, 64, 128)
SIZES = (128, 64, 32, 16, 8)
HWS = tuple(s * s for s in SIZES)          # (16384, 4096, 1024, 256, 64)
B, C, NBOX = 8, 256, 16
N_CORES = 8
N_LEVELS = 5

# Static block grid: per level the host lays the permuted pixel stream
# out as [masked | pad | unmasked | pad | zeros] over BLK_N blocks.
BLK_W = (512, 512, 512, 256, 64)           # block width per level
BLK_N = (33, 9, 3, 2, 2)                   # blocks per level
BLOCKS = [(l, BLK_W[l]) for l in range(N_LEVELS) for _ in range(BLK_N[l])]
NBLK = len(BLOCKS)                         # 49
LVL_B0 = [sum(BLK_N[:l]) for l in range(N_LEVELS)]

# DMA chunks: groups of up to 4 consecutive blocks -> one dram tensor /
# one ~1MB DMA each.  (chunk_id, [block ids], [offsets], total_w)
CHUNKS = []
_i = 0
while _i < NBLK:
    ids = list(range(_i, min(_i + 4, NBLK)))
    offs = []
    tw = 0
    for k in ids:
        offs.append(tw)
        tw += BLOCKS[k][1]
    CHUNKS.append((len(CHUNKS), ids, offs, tw))
    _i = ids[-1] + 1

# Per-block route: "A"  = PE negI subtract + ACT Square from psum
#                  "Ba" = DVE stt subtract + ACT Square from bf16 d
#                  "Bd" = DVE stt subtract + DVE square from bf16 d
_CYCLE = ("A", "A", "Bd")
ROUTES = [_CYCLE[i % len(_CYCLE)] for i in range(NBLK)]


def _build_module():
    import concourse.mybir as mybir
    from concourse import bacc
    from concourse.tile import TileContext

    dt = mybir.dt
    nc = bacc.Bacc("TRN2", target_bir_lowering=False, debug=False,
                   num_devices=N_CORES)

    ch_d = [nc.dram_tensor(f"ch{c}", [128, 4, tw], dt.float8e4,
                           kind="ExternalInput")
            for (c, _, _, tw) in CHUNKS]
    wt_d = nc.dram_tensor("wt", [128, 4 * N_LEVELS, 128], dt.float8e4,
                          kind="ExternalInput")
    ni_d = nc.dram_tensor("ni", [128, 128], dt.float8e4, kind="ExternalInput")
    out_q = nc.dram_tensor("out_q", [128, NBLK], dt.float32,
                           kind="ExternalOutput")

    SUB = mybir.AluOpType.subtract
    MULT = mybir.AluOpType.mult
    SQUARE = mybir.ActivationFunctionType.Square
    DR = mybir.MatmulPerfMode.DoubleRow

    with TileContext(nc) as tc:
        with (
            tc.tile_pool(name="const", bufs=1) as const_pool,
            tc.tile_pool(name="feat", bufs=4) as feat_pool,
            tc.tile_pool(name="work", bufs=4) as work_pool,
            tc.tile_pool(name="ps", bufs=4, space="PSUM") as psum_pool,
        ):
            wt = const_pool.tile([128, 4 * N_LEVELS, 128], dt.float8e4)
            ni = const_pool.tile([128, 128], dt.float8e4)
            qcat = const_pool.tile([128, NBLK], dt.float32)
            garb_a = const_pool.tile([128, 1024], dt.bfloat16)
            garb_v = const_pool.tile([128, 1024], dt.bfloat16)

            nc.sync.dma_start(out=wt[:], in_=wt_d[:])
            nc.sync.dma_start(out=ni[:], in_=ni_d[:])

            for (cid, ids, offs, tw) in CHUNKS:
                ch = feat_pool.tile([128, 4, tw], dt.float8e4, tag="ch")
                nc.sync.dma_start(out=ch[:], in_=ch_d[cid][:])

                for k, boff in zip(ids, offs):
                    lvl, w = BLOCKS[k]
                    route = ROUTES[k]
                    ps = psum_pool.tile([128, 1024], dt.float32, tag="ps")
                    # adaptation matmuls (one DoubleRow K=256 per oc chunk)
                    for j in range(2):
                        nc.tensor.matmul(
                            ps[:, j * w:(j + 1) * w],
                            wt[:, 4 * lvl + 2 * j:4 * lvl + 2 * j + 2, :],
                            ch[:, 0:2, boff:boff + w],
                            start=True, stop=(route != "A"), perf_mode=DR)
                    if route == "A":
                        # psum -= 16 * t'' via -16I matmul
                        for j in range(2):
                            nc.tensor.matmul(
                                ps[:, j * w:(j + 1) * w],
                                ni[:],
                                ch[:, 2 + j, boff:boff + w],
                                start=False, stop=True)
                        nc.scalar.activation(
                            garb_a[:, 0:2 * w], ps[:, 0:2 * w], SQUARE,
                            scale=1.0 / 256.0, accum_out=qcat[:, k:k + 1])
                    else:
                        d = work_pool.tile([128, 1024], dt.bfloat16, tag="d")
                        # d = 16*t'' - psum  (= 256*(t' - a))
                        nc.vector.scalar_tensor_tensor(
                            d[:, 0:2 * w], ch[:, 2:4, boff:boff + w], S_SCALE,
                            ps[:, 0:2 * w], op0=MULT, op1=SUB)
                        if route == "Ba":
                            nc.scalar.activation(
                                garb_a[:, 0:2 * w], d[:, 0:2 * w], SQUARE,
                                scale=1.0 / 256.0, accum_out=qcat[:, k:k + 1])
                        else:
                            nc.vector.scalar_tensor_tensor(
                                garb_v[:, 0:2 * w], d[:, 0:2 * w],
                                1.0 / 65536.0, d[:, 0:2 * w],
                                op0=MULT, op1=MULT,
                                accum_out=qcat[:, k:k + 1])

            nc.sync.dma_start(out=out_q[:], in_=qcat[:])

    nc.compile()
    return nc


def _rasterize_masks(gt_bboxes):
    """Host-side mask rasterization, mirroring reference.gt_mask.

    Returns per-level [B, HW] bool masks."""
    out = []
    for lvl in range(N_LEVELS):
        h = w = SIZES[lvl]
        stride = np.float32(STRIDES[lvl])
        q = np.floor(gt_bboxes.astype(np.float32) / stride).astype(np.int32)
        lx = np.minimum(q[..., 0], w - 1)
        ly = np.minimum(q[..., 1], h - 1)
        rx = np.minimum(q[..., 2], w - 1)
        ry = np.minimum(q[..., 3], h - 1)
        lm = np.zeros((B, h * w), bool)
        for b in range(B):
            m = np.zeros((h, w), bool)
            for i in range(gt_bboxes.shape[1]):
                if lx[b, i] == rx[b, i] or ly[b, i] == ry[b, i]:
                    m[ly[b, i], lx[b, i]] = True
                else:
                    m[ly[b, i]:ry[b, i], lx[b, i]:rx[b, i]] = True
            lm[b] = m.reshape(-1)
        out.append(lm)
    return out


_NC_CACHE = None


def _get_nc():
    global _NC_CACHE
    if _NC_CACHE is None:
        _NC_CACHE = _build_module()
    return _NC_CACHE


def _run(in_maps, trace=False, trace_cores=None):
    from concourse.bass_utils import run_bass_kernel_spmd

    kwargs = {}
    if trace:
        kwargs.update(trace=True, trace_cores=trace_cores or [0])
    return run_bass_kernel_spmd(_get_nc(), in_maps, core_ids=list(range(N_CORES)),
                                **kwargs)


def _pack_wt(inputs):
    """wt[p, 4l+2j+i, m] = 16 * W_l[128j + m, 128i + p]."""
    wtp = np.zeros((128, 4 * N_LEVELS, 128), np.float32)
    for lvl in range(N_LEVELS):
        w = np.asarray(inputs[f"adapt_w{lvl}"], np.float32)
        for j in range(2):
            for i in range(2):
                wtp[:, 4 * lvl + 2 * j + i, :] = \
                    w[128 * j:128 * j + 128, 128 * i:128 * i + 128].T
    return (wtp * S_SCALE).astype(F8)


def _prep_in_maps(inputs, masks):
    """Per-core chunk arrays [128, 4, CW]: [s_ic0 | s_ic1 | t_oc0 | t_oc1],
    pixels permuted mask-first and zero-padded to the static block grid.
    Returns (in_maps, mblocks[B][L]) where mblocks = #masked blocks."""
    wtp = _pack_wt(inputs)
    negi = (-S_SCALE * np.eye(128, dtype=np.float32)).astype(F8)
    mblocks = [[0] * N_LEVELS for _ in range(N_CORES)]
    in_maps = []
    for b in range(N_CORES):
        m = {"wt": wtp, "ni": negi}
        # full padded streams per level, then slice into chunks
        lvl_st = []
        for lvl in range(N_LEVELS):
            hw, g, nb = HWS[lvl], BLK_W[lvl], BLK_N[lvl]
            s = np.asarray(inputs[f"feat_s{lvl}"][b], np.float32).reshape(C, hw)
            bv = np.asarray(inputs[f"adapt_b{lvl}"], np.float32)
            t = np.asarray(inputs[f"feat_t{lvl}"][b], np.float32).reshape(C, hw)
            tp = t - bv[:, None]
            mask = masks[lvl][b]
            midx = np.flatnonzero(mask)
            uidx = np.flatnonzero(~mask)
            nm = len(midx)
            mb = -(-nm // g)                      # ceil
            mblocks[b][lvl] = mb
            tot = nb * g
            st = np.zeros((128, 4, tot), np.float32)
            for (arr, base) in ((s, 0), (tp, 2)):
                sc = arr * S_SCALE
                pm = sc[:, midx]
                pu = sc[:, uidx]
                st[:, base + 0, 0:nm] = pm[0:128]
                st[:, base + 1, 0:nm] = pm[128:256]
                st[:, base + 0, mb * g:mb * g + len(uidx)] = pu[0:128]
                st[:, base + 1, mb * g:mb * g + len(uidx)] = pu[128:256]
            lvl_st.append(st.astype(F8))
        # global per-block pixel stream -> chunk tensors
        blk_arr = []
        for k, (lvl, w) in enumerate(BLOCKS):
            i = k - LVL_B0[lvl]
            blk_arr.append(lvl_st[lvl][:, :, i * w:(i + 1) * w])
        for (cid, ids, offs, tw) in CHUNKS:
            m[f"ch{cid}"] = np.ascontiguousarray(
                np.concatenate([blk_arr[k] for k in ids], axis=2))
        in_maps.append(m)
    return in_maps, mblocks


def kernel(_trace=False, _return_results=False, **inputs):
    gt_bboxes = np.asarray(inputs["gt_bboxes"], np.float32)
    masks = _rasterize_masks(gt_bboxes)
    in_maps, mblocks = _prep_in_maps(inputs, masks)

    res = _run(in_maps, trace=_trace)

    loss = np.float64(0.0)
    for lvl in range(N_LEVELS):
        s_gt = np.float64(0.0)
        s_bg = np.float64(0.0)
        k0, nb = LVL_B0[lvl], BLK_N[lvl]
        for c in range(N_CORES):
            q = res.results[c]["out_q"].astype(np.float64)
            qb = q[:, k0:k0 + nb].sum(axis=0)
            mb = mblocks[c][lvl]
            s_gt += qb[:mb].sum()
            s_bg += qb[mb:].sum()
        loss += WEIGHT_GT * np.sqrt(s_gt + 1e-8) + \
            WEIGHT_BG * np.sqrt(s_bg + 1e-8)

    out = np.array(loss, dtype=np.float32)
    if _return_results:
        return out, res
    return out
